# revision 2
# baseline (speedup 1.0000x reference)
"""ROI max-pooling (B=2, N=64, C=256, H=W=64, 7x7 out) on 8 TRN2 cores — v3.

Design:
- Host converts conv_out to fp16 and pre-slices a per-core row band
  (uniform R_MAX rows across cores), so the input DMA program is
  identical on every core: branch-free, issued immediately at kernel
  start on the two HWDGE queues (Sync + Scalar).
- Core = b*4 + nh*2 + cg (batch, roi-half, channel group): the 4-way
  body index is pid >> 1. Only the per-roi window APs branch.
- The roi halves per batch are chosen by y-clustering + compute balance
  (local search): fewer rows loaded AND balanced DVE time.
- All per-roi reduce_max windows run on the DVE in fp16 (TRN2's Pool/
  Scalar engines cannot do free-axis reductions). Windows are baked
  into APs at trace time.
- Output staged fp16 (max of fp16 inputs is exact in fp16), written
  back in completion-ordered slices alternating between Sync and
  Scalar, upconverted to f32 on the host.
"""

import os

os.environ.setdefault("MYCRO_LOCAL_CACHE", "1")

import numpy as np

B, N, C, H, W = 2, 64, 256, 64, 64
POOL_H = POOL_W = 7
ANCHOR_STRIDE = 16
N_CORES = 8
N_PER_CORE = N // 2  # 32
CELLS = POOL_H * POOL_W  # 49
N_CHUNKS = 4

# ---- fitted cost model (ns) — refit from probe dumps ----------------------
# measured on HW (probe run 8), fp16, ns
_DVE_COST = {
    (1, 1): 110, (1, 2): 118, (1, 3): 213, (1, 4): 263,
    (2, 1): 120, (2, 2): 263, (2, 3): 365, (2, 4): 467,
    (3, 1): 253, (3, 2): 365, (3, 3): 519, (3, 4): 672,
    (4, 1): 396, (4, 2): 469, (4, 3): 673, (4, 4): 877,
}
# "tt" (single tensor_tensor max) is only legal for K=2 windows: longer
# chains have an unsynchronized same-engine RAW hazard on the DVE.
_DVE_VARIANT = {
    (1, 1): "rc", (1, 2): "tt", (1, 3): "rc", (1, 4): "rc",
    (2, 1): "tt", (2, 2): "rc", (2, 3): "rc", (2, 4): "cr",
    (3, 1): "cr", (3, 2): "rc", (3, 3): "rc", (3, 4): "rc",
    (4, 1): "x3c", (4, 2): "rc", (4, 3): "rc", (4, 4): "rc",
}

# schedule model constants (ns)
_ROW_NS = 45.0  # fp16 row (128p x 64 x 2B = 16 KiB) transfer
_T_ISSUE = 700.0
_T_SEM = 500.0
_T_START = 900.0
_T_SETUP = 2100.0  # DVE reg+branch before first reduce


def _cost_dve(kh, kw):
    v = _DVE_COST.get((kh, kw))
    if v is None:
        v = int(130 + 0.55 * CELLS * kh * kw)
    return v


def _split_cost(kh, kw, sw):
    """Two-stage form: (kh-1) vertical region-tts + horizontal stage."""
    if kh < 2:
        return None
    wext = 6 * sw + kw
    if wext > 64:
        return None
    vert = (kh - 1) * (62.0 + 4.1 * wext)
    if kw == 1:
        horiz = 110.0
    elif kw == 2:
        horiz = 120.0
    else:
        horiz = 59.0 + 49 * kw * 1.05
    return vert + horiz


def _roi_form(kh, kw, sw):
    d = _cost_dve(kh, kw)
    s = _split_cost(kh, kw, sw)
    if s is not None and s < d - 20:
        return "split", int(s)
    return "direct", d


# ---- roi params ------------------------------------------------------------
def _expand(lo, hi, pool, limit):
    for _ in range(pool):
        need = (hi - lo + 1) < pool
        lo = np.where(need, np.maximum(0, lo - 1), lo)
        hi = np.where(need, np.minimum(limit - 1, hi + 1), hi)
    return lo, hi


def _roi_params(rois: np.ndarray):
    coords = (np.asarray(rois, np.float32) / ANCHOR_STRIDE).astype(np.int32)
    x1, y1, x2, y2 = (coords[..., i] for i in range(4))
    y1, y2 = _expand(y1, y2, POOL_H, H)
    x1, x2 = _expand(x1, x2, POOL_W, W)
    rh = y2 - y1 + 1
    rw = x2 - x1 + 1
    kh = -(-rh // POOL_H)
    sh = rh // POOL_H
    kw = -(-rw // POOL_W)
    sw = rw // POOL_W
    return y1, x1, sh, sw, kh, kw


# ---- planning --------------------------------------------------------------
def _row_extent(params, b, n):
    y1, x1, sh, sw, kh, kw = params
    lo = int(y1[b, n])
    hi = lo + 6 * int(sh[b, n]) + int(kh[b, n]) - 1
    return lo, hi


def _chunk_bounds(r_max):
    """Row-boundary list per chunk: first chunk smaller for early start."""
    c0 = max(4, r_max // 6)
    rest = r_max - c0
    sizes = [c0] + [rest // 3] * 2 + [rest - 2 * (rest // 3)]
    bounds = []
    acc = 0
    for s in sizes:
        acc += s
        bounds.append(acc)
    return bounds  # cumulative end-rows


def _land_times(r_max):
    """Modeled landing time per chunk (two HWDGE queues, shared BW)."""
    bounds = _chunk_bounds(r_max)
    sizes = [bounds[0]] + [bounds[i] - bounds[i - 1] for i in range(1, N_CHUNKS)]
    # c0+c1 start together (scalar/sync queues), share bandwidth
    t = _T_START + _T_ISSUE
    land = [0.0] * N_CHUNKS
    land[0] = t + (sizes[0] + min(sizes[0], sizes[1] // 2)) * _ROW_NS + _T_SEM
    land[1] = t + (sizes[0] + sizes[1]) * _ROW_NS + _T_SEM
    land[2] = t + (sizes[0] + sizes[1] + sizes[2]) * _ROW_NS + _T_SEM
    land[3] = t + r_max * _ROW_NS + _T_SEM
    return land


def _chunk_of(row, rmin, r_max):
    bounds = _chunk_bounds(r_max)
    rel = min(max(row - rmin, 0), r_max - 1)
    for c, e in enumerate(bounds):
        if rel < e:
            return c
    return N_CHUNKS - 1


def _sched_core(params, b, rois, rmin, r_max):
    """DVE schedule with chunk-landing ready times.
    Returns (makespan, order)."""
    y1, x1, sh, sw, kh, kw = params
    land = _land_times(r_max)
    items = []
    for n in rois:
        lo, hi = _row_extent(params, b, n)
        ready = land[_chunk_of(hi, rmin, r_max)]
        items.append(
            (ready, n, _roi_form(int(kh[b, n]), int(kw[b, n]), int(sw[b, n]))[1])
        )
    items.sort(key=lambda t: (t[0], -t[2]))
    clk = _T_SETUP
    order = []
    for ready, n, cd in items:
        clk = max(clk, ready) + cd
        order.append(n)
    return clk, order


def _plan_halves(params, b):
    ext = [_row_extent(params, b, n) for n in range(N)]
    order = sorted(range(N), key=lambda n: ext[n][0] + ext[n][1])
    halves = [list(order[:N_PER_CORE]), list(order[N_PER_CORE:])]

    def score(hs):
        spans = []
        for h in hs:
            lo = min(ext[n][0] for n in h)
            hi = max(ext[n][1] for n in h)
            spans.append((lo, hi))
        r_used = max(hi - lo + 1 for lo, hi in spans)
        mks = []
        for h, (lo, hi) in zip(hs, spans):
            rmin = max(0, min(lo, H - r_used))
            mk, _ = _sched_core(params, b, h, rmin, r_used)
            mks.append(mk)
        # small row tiebreak: fewer rows -> less DMA, breaks plateau drift
        return max(mks) + 4.0 * r_used

    cur = score(halves)
    rng = np.random.default_rng(0)
    for _ in range(400):
        i = int(rng.integers(0, N_PER_CORE))
        j = int(rng.integers(0, N_PER_CORE))
        halves[0][i], halves[1][j] = halves[1][j], halves[0][i]
        new = score(halves)
        if new <= cur:
            cur = new
        else:
            halves[0][i], halves[1][j] = halves[1][j], halves[0][i]
    return halves


def _plan(params):
    halves_by_b = [_plan_halves(params, b) for b in range(B)]
    exts = []
    for b in range(B):
        for h in halves_by_b[b]:
            lo = min(_row_extent(params, b, n)[0] for n in h)
            hi = max(_row_extent(params, b, n)[1] for n in h)
            exts.append((lo, hi))
    r_max = max(hi - lo + 1 for lo, hi in exts)
    r_max = min(H, -(-r_max // 4) * 4)

    bodies = []
    for j in range(4):
        b, nh = j >> 1, j & 1
        h = halves_by_b[b][nh]
        lo, hi = exts[j]
        rmin = max(0, min(lo, H - r_max))
        mk, order = _sched_core(params, b, h, rmin, r_max)
        slots = {n: i for i, n in enumerate(order)}
        bodies.append(
            dict(b=b, nh=nh, rois=h, rmin=rmin, order=order, slots=slots, mk=mk)
        )
    return bodies, r_max


# ---- device program --------------------------------------------------------
def _build_nc(params):
    import contextlib

    import concourse.bass as bass
    from concourse import mybir

    y1, x1, sh, sw, kh, kw = params
    f16 = mybir.dt.float16

    bodies, r_max = _plan(params)
    FS = r_max * W
    OS = N_PER_CORE * CELLS
    bounds = _chunk_bounds(r_max)
    starts = [0] + bounds[:-1]

    # visit branch bodies most-expensive first (cheapest branch-skip cost
    # for the core that determines the makespan)
    branch_order = sorted(range(4), key=lambda j: -bodies[j]["mk"])

    nc = bass.Bass()
    conv = nc.declare_dram_parameter("conv", [128, FS], f16, isOutput=False)
    out = nc.declare_dram_parameter("out", [128, OS], f16, isOutput=True)

    with contextlib.ExitStack() as ctx:
        slab = ctx.enter_context(nc.sbuf_tensor("slab", [128, FS], f16))
        ostage = ctx.enter_context(nc.sbuf_tensor("ostage", [128, OS], f16))
        tmps = [
            ctx.enter_context(nc.sbuf_tensor(f"tmp{i}", [128, 7 * 64], f16))
            for i in range(4)
        ]
        spbuf = ctx.enter_context(nc.sbuf_tensor("spbuf", [128, 16], f16))
        chunk_sems = [
            ctx.enter_context(nc.semaphore(f"chunk{c}")) for c in range(N_CHUNKS)
        ]
        vsem = ctx.enter_context(nc.semaphore("vsem"))
        osem = ctx.enter_context(nc.semaphore("osem"))
        block = ctx.enter_context(nc.Block())

        sl = slab[:]
        slab_t = sl.tensor
        part_pair = list(sl.ap[0])

        def body_idx_reg(eng):
            pid = eng.alloc_register("pid")
            idx = eng.alloc_register("idx")
            eng.reg_load(pid, nc.partition_id_tensor[0:1, 0:1])
            eng.reg_div(idx, pid, 2)
            return idx

        def emit_roi(eng, b, n, rmin, slot):
            _kh, _kw = int(kh[b, n]), int(kw[b, n])
            base = sl.offset + (int(y1[b, n]) - rmin) * W + int(x1[b, n])
            dims3 = [
                part_pair,
                [int(sh[b, n]) * W, POOL_H],
                [int(sw[b, n]), POOL_W],
            ]
            out_ap = ostage[:, slot * CELLS : (slot + 1) * CELLS]
            v = _DVE_VARIANT.get((_kh, _kw), "rc")
            if _kh == 1 and _kw == 1:
                return eng.tensor_copy(out_ap, bass.AP(slab_t, base, dims3))
            if v == "tt":
                wins = [
                    (dr, dc) for dr in range(_kh) for dc in range(_kw)
                ]

                def wap(dr, dc):
                    return bass.AP(slab_t, base + dr * W + dc, dims3)

                inst = eng.tensor_tensor(
                    out_ap, wap(*wins[0]), wap(*wins[1]), op=mybir.AluOpType.max
                )
                for wv in wins[2:]:
                    inst = eng.tensor_tensor(
                        out_ap, out_ap, wap(*wv), op=mybir.AluOpType.max
                    )
                return inst
            if v == "x3c" and _kw == 1:
                return eng.reduce_max(
                    out_ap,
                    bass.AP(slab_t, base, dims3 + [[W, _kh]]),
                    axis=mybir.AxisListType.X,
                )
            rpair = [[W, _kh], [1, _kw]]
            if v == "cr":
                rpair = [rpair[1], rpair[0]]
            return eng.reduce_max(
                out_ap,
                bass.AP(slab_t, base, dims3 + rpair),
                axis=mybir.AxisListType.XY,
            )

        def marks_for(k):
            """4 progress marks -> 4 output slices, last slices small."""
            if k == 0:
                return []
            ms = [int(k * 0.45), int(k * 0.72), k - 2, k]
            return sorted(set(max(1, min(k, m)) for m in ms))

        body_marks = [marks_for(len(bd["order"])) for bd in bodies]

        def chunk_dma(eng, c):
            eng.dma_start(
                slab[:, starts[c] * W : bounds[c] * W],
                conv[:, starts[c] * W : bounds[c] * W],
            ).then_inc(chunk_sems[c], 16)

        def emit_out_slices(eng, parity):
            idx = body_idx_reg(eng)
            # critical body LAST: its core pays no exit-walk before drain
            for j in reversed(branch_order):
                bd = bodies[j]
                mk = body_marks[j]
                with eng.If_eq(idx, j):
                    prev = 0
                    for k, m in enumerate(mk):
                        lo_s, hi_s = prev, m
                        prev = m
                        if k % 2 != parity:
                            continue
                        eng.wait_ge(vsem, k + 1)
                        eng.dma_start(
                            out[:, lo_s * CELLS : hi_s * CELLS],
                            ostage[:, lo_s * CELLS : hi_s * CELLS],
                        ).then_inc(osem, 16)

        @block.scalar
        def _(scalar):
            chunk_dma(scalar, 1)
            chunk_dma(scalar, 3)
            emit_out_slices(scalar, 0)

        @block.sync
        def _(sync):
            chunk_dma(sync, 0)
            chunk_dma(sync, 2)
            emit_out_slices(sync, 1)

        def roi_steps(b, n, rmin, slot):
            """List of emit closures; consecutive steps of one roi are
            data-dependent and must not be emitted back-to-back."""
            _kh, _kw = int(kh[b, n]), int(kw[b, n])
            _sh, _sw = int(sh[b, n]), int(sw[b, n])
            form, _ = _roi_form(_kh, _kw, _sw)
            if form == "direct":
                return [
                    lambda eng, tmp, b=b, n=n, rmin=rmin, slot=slot: emit_roi(
                        eng, b, n, rmin, slot
                    )
                ]
            base = sl.offset + (int(y1[b, n]) - rmin) * W + int(x1[b, n])
            wext = 6 * _sw + _kw
            out_ap = ostage[:, slot * CELLS : (slot + 1) * CELLS]
            steps = []

            def mk_v(i):
                def f(eng, tmp):
                    dst = tmp[:, 0 : 7 * wext]
                    src_i = bass.AP(
                        slab_t, base + i * W, [part_pair, [_sh * W, 7], [1, wext]]
                    )
                    if i == 1:
                        src_0 = bass.AP(
                            slab_t, base, [part_pair, [_sh * W, 7], [1, wext]]
                        )
                        return eng.tensor_tensor(
                            dst, src_0, src_i, op=mybir.AluOpType.max
                        )
                    return eng.tensor_tensor(dst, dst, src_i, op=mybir.AluOpType.max)

                return f

            for i in range(1, _kh):
                steps.append(mk_v(i))

            def h(eng, tmp):
                tap = tmp[:]
                tpart = list(tap.ap[0])
                if _kw == 2:
                    a0 = bass.AP(tap.tensor, tap.offset, [tpart, [wext, 7], [_sw, 7]])
                    a1 = bass.AP(
                        tap.tensor, tap.offset + 1, [tpart, [wext, 7], [_sw, 7]]
                    )
                    return eng.tensor_tensor(out_ap, a0, a1, op=mybir.AluOpType.max)
                if _kw == 1:
                    return eng.tensor_copy(
                        out_ap,
                        bass.AP(tap.tensor, tap.offset, [tpart, [wext, 7], [_sw, 7]]),
                    )
                return eng.reduce_max(
                    out_ap,
                    bass.AP(
                        tap.tensor,
                        tap.offset,
                        [tpart, [wext, 7], [_sw, 7], [1, _kw]],
                    ),
                    axis=mybir.AxisListType.X,
                )

            steps.append(h)
            return steps

        @block.vector
        def _(vector):
            idx = body_idx_reg(vector)
            # critical body FIRST: its core pays no entry-walk before compute
            for j in branch_order:
                bd = bodies[j]
                mk = body_marks[j]
                with vector.If_eq(idx, j):
                    waited = set()
                    order = bd["order"]
                    done = 0
                    mark_i = 0
                    pair_idx = 0
                    ptr = 0
                    while ptr < len(order):
                        pair = order[ptr : ptr + 2]
                        ptr += len(pair)
                        need_c = 0
                        for n in pair:
                            lo, hi = _row_extent(params, bd["b"], n)
                            need_c = max(need_c, _chunk_of(hi, bd["rmin"], r_max))
                        for cc in range(need_c + 1):
                            if cc not in waited:
                                vector.wait_ge(chunk_sems[cc], 16)
                                waited.add(cc)
                        streams = [
                            roi_steps(bd["b"], n, bd["rmin"], bd["slots"][n])
                            for n in pair
                        ]
                        tmpsel = [tmps[(2 * pair_idx + i) % 4] for i in range(2)]
                        pair_idx += 1
                        # interleave; spacer when a chain would self-follow
                        idxs = [0] * len(streams)
                        last_src = -1
                        last_inst = None
                        while any(
                            idxs[s] < len(streams[s]) for s in range(len(streams))
                        ):
                            cands = [
                                s
                                for s in range(len(streams))
                                if idxs[s] < len(streams[s]) and s != last_src
                            ]
                            if not cands:
                                # only the same chain remains: spacer
                                vector.tensor_copy(spbuf[:, 0:8], spbuf[:, 8:16])
                                last_src = -1
                                continue
                            s = max(cands, key=lambda q: len(streams[q]) - idxs[q])
                            last_inst = streams[s][idxs[s]](vector, tmpsel[s])
                            idxs[s] += 1
                            last_src = s
                        done += len(pair)
                        incs = 0
                        while mark_i < len(mk) and done >= mk[mark_i]:
                            incs += 1
                            mark_i += 1
                        if incs:
                            last_inst.then_inc(vsem, incs)

    return nc, bodies, r_max


_CACHE: dict[bytes, object] = {}
LAST_RESULT = None
LAST_PLAN = None


def _get_built(params_key: bytes, params):
    built = _CACHE.get(params_key)
    if built is None:
        built = _build_nc(params)
        _CACHE[params_key] = built
    return built


def kernel(rois: np.ndarray, conv_out: np.ndarray) -> np.ndarray:
    from concourse.bass_utils import run_bass_kernel_spmd

    rois = np.asarray(rois)
    conv_out = np.asarray(conv_out, np.float32)
    params = _roi_params(rois)
    params_key = b"".join(np.ascontiguousarray(p).tobytes() for p in params)
    nc, bodies, r_max = _get_built(params_key, params)
    global LAST_PLAN
    LAST_PLAN = (bodies, r_max)

    in_maps = []
    for core in range(N_CORES):
        b, nh, cg = core >> 2, (core >> 1) & 1, core & 1
        bd = bodies[b * 2 + nh]
        rmin = bd["rmin"]
        sl = conv_out[b, cg * 128 : (cg + 1) * 128, rmin : rmin + r_max, :]
        in_maps.append(
            {"conv": np.ascontiguousarray(sl, dtype=np.float16).reshape(128, -1)}
        )

    res = run_bass_kernel_spmd(nc, in_maps, list(range(N_CORES)))
    global LAST_RESULT
    LAST_RESULT = res

    out = np.empty((B, N, C, POOL_H, POOL_W), np.float32)
    for core in range(N_CORES):
        b, nh, cg = core >> 2, (core >> 1) & 1, core & 1
        bd = bodies[b * 2 + nh]
        r = (
            res.results[core]["out"]
            .reshape(128, N_PER_CORE, CELLS)
            .astype(np.float32)
        )
        for n in bd["rois"]:
            s = bd["slots"][n]
            out[b, n, cg * 128 : (cg + 1) * 128] = r[:, s].reshape(
                128, POOL_H, POOL_W
            )
    return out
